# revision 35
# baseline (speedup 1.0000x reference)
"""Cost-adjusted cross-entropy loss on 8 Trainium2 NeuronCores (Bass/Tile).

Math (per sample i of N=65536, C=1000 classes):
    t_i   = super_classes[target_i]
    p_i   = argmax_c logits[i, c]
    w_i   = class_weights[t_i, p_i]
    lse_i = log(sum_c exp(logits[i, c]))        # logits ~ N(0,1): no max-shift needed
    l2_i  = w_i * (lse_i - logits[i, t_i])      # == -w_i * log_softmax(logits)[i, t_i]
    out   = sum(l2) / sum(w)

Sharding: data-parallel over N. Each core gets 8192 rows of logits/target;
class_weights and super_classes are replicated. Per-core kernel emits [128, 2]
per-partition partial (numerator, denominator) sums; the host adds the 8x128
partials and divides.

Per-core device plan (v3 — zero-cost packed argmax, one DVE pass per tile):
  - target/super_classes arrive as int32 pairs (host reinterprets the int64
    bytes; values < 1000 so the low words carry everything).
  - t_i, logits[i, t_i] and class_weights[t_i, p_i] are fetched with
    indirect-DMA element gathers (8192 descriptors each) on the Pool engine.
  - Packed-argmax trick: each int32 word of a packed tile holds
    (f16(exp(x)) << 16) | column. exp(x) > 0, so the f16 bit pattern in the
    high half is order-preserving, and one fp32 reduce_max per tile yields
    max value AND argmax (low 16 bits), with only a 2^-11 relative tie-fuzz.
    The packing itself is free: the column iota is pre-written into the low
    u16 lanes of three rotating buffers, and ScalarE's exp writes f16
    results directly into the high u16 lanes via a strided AP.
  - Logits are downcast to bf16 on the host (memory-bound kernel: halves
    HBM traffic; quantization is ~100x inside the tolerance).
  - The streaming loop (64 tiles of [128 rows, 1000]):
      DMA load    : HWDGE via nc.sync (SP), bf16 tiles
      ScalarE     : exp -> f16 high lanes, fp32 row-sum accum -> sumexp
      VectorE     : single max8 pass over the packed tile (fp32 view)
  - Tail (per eighth/quarter, overlapped with the next chunk's stream):
    argmax = low 16 bits of packed max, w gather, ln(sumexp),
    w*(lse-g) partial sums; final combine + DMA out.
"""

import numpy as np

import concourse.bass as bass
import concourse.bacc as bacc
import concourse.tile as tile
from concourse import mybir
from concourse.bass_utils import run_bass_kernel_spmd
from concourse.hw_specs import get_activation_tables

N, C = 65536, 1000
NCORES = 8
R = N // NCORES          # rows per core
P = 128                  # partitions
TILES = R // P           # row-tiles per core

F32 = mybir.dt.float32
F16 = mybir.dt.float16
BF16 = mybir.dt.bfloat16
I32 = mybir.dt.int32
U16 = mybir.dt.uint16
AX = mybir.AxisListType.X
ALU = mybir.AluOpType
AF = mybir.ActivationFunctionType
NPKBUF = 3               # rotating packed-tile buffers


def _stt_int(nc, eng, out, in0, imm, in1, op0, op1):
    """scalar_tensor_tensor with an int32-typed immediate: walrus requires
    bitvec-op immediates to be integer-typed and match src/dst dtype, but
    the bass API hardcodes float32 immediates."""
    return eng.add_instruction(
        mybir.InstTensorScalarPtr(
            name=nc.get_next_instruction_name(),
            is_scalar_tensor_tensor=True,
            op0=op0,
            op1=op1,
            ins=[
                eng.lower_ap(in0),
                mybir.ImmediateValue(dtype=mybir.dt.int32, value=imm),
                eng.lower_ap(in1),
            ],
            outs=[eng.lower_ap(out)],
        )
    )


def build_kernel(tc, x, t32, sc32, cw, out):
    nc = tc.nc
    with (  # noqa
        tc.tile_pool(name="singles", bufs=1) as singles,
        tc.tile_pool(name="xp", bufs=4) as xp,
    ):
        # Persistent per-sample accumulators, laid out [partition, tile].
        sumexp_all = singles.tile([P, TILES], F32)
        v8_all = singles.tile([P, TILES, 8], F32)
        tgt2 = singles.tile([P, TILES, 2], I32)
        tgt = singles.tile([P, TILES], I32)
        tlo = singles.tile([P, TILES], I32)
        rowbase = singles.tile([P, TILES], I32)
        goff = singles.tile([P, TILES], I32)
        g = singles.tile([P, TILES], BF16)
        pidx = singles.tile([P, TILES], I32)
        woff = singles.tile([P, TILES], I32)
        w = singles.tile([P, TILES], F32)
        lse = singles.tile([P, TILES], F32)
        diff = singles.tile([P, TILES], F32)
        prod = singles.tile([P, TILES], F32)
        partials = singles.tile([P, 2], F32)

        # Rotating packed tiles: low u16 lanes hold the column iota 0..999
        # (written once); ScalarE writes f16 exp values into the high lanes.
        pks = [
            singles.tile([P, C], I32, name=f"pk{i}") for i in range(NPKBUF)
        ]
        pk16s = [
            pk[:].bitcast(U16).rearrange("p (c two) -> p c two", two=2)
            for pk in pks
        ]
        for pk16 in pk16s:
            nc.gpsimd.iota(
                pk16[:, :, 0], pattern=[[1, C]], base=0, channel_multiplier=0
            )

        # Pre-load the one act-function set containing BOTH Exp and Ln, so
        # the automatic table-load pass doesn't bounce between the exp-only
        # and ln-only sets (~1.3us per switch). Best-effort: fall back to
        # automatic placement if the act tables can't be resolved.
        try:
            tabs = get_activation_tables(nc.m.arch)
            combined = [
                i for i, fns in enumerate(tabs.values())
                if AF.Exp in fns and AF.Ln in fns
            ]
        except Exception:
            combined = []
        if combined:
            nc.scalar.add_instruction(
                mybir.InstLoadActFuncSet(
                    name=nc.get_next_instruction_name(),
                    ins=[],
                    outs=[],
                    act_func_set_id=combined[0],
                )
            )

        # --- Upfront index plumbing (overlaps the streaming loop) ---------
        # tgt[p, t] = target[128 t + p]  (low int32 word of the int64).
        nc.gpsimd.dma_start(
            out=tgt2[:],
            in_=t32.rearrange("(t p) two -> p t two", p=P),
        )
        # tgt = 2 * low_word(target): element offset of the low int32 word of
        # super_classes[target] in the flat int32 view.  (Multi-element-per-
        # index indirect gathers are broken on HW; single-element gathers are
        # exact, so gather just the low words.)
        nc.vector.tensor_scalar(
            out=tgt[:], in0=tgt2[:, :, 0], scalar1=2, scalar2=None, op0=ALU.mult
        )
        # tlo[p, t] = super_classes[target[...]]  (low int32 word)
        nc.gpsimd.indirect_dma_start(
            out=tlo[:],
            out_offset=None,
            in_=sc32.rearrange("a b -> (a b)")[:, None],
            in_offset=bass.IndirectOffsetOnAxis(ap=tgt[:], axis=0),
        )
        # rowbase[p, t] = (128 t + p) * 1000   (iota steps are int16-limited,
        # so generate the row index first and scale on the vector engine)
        nc.gpsimd.iota(
            rowbase[:], pattern=[[P, TILES]], base=0, channel_multiplier=1
        )
        nc.vector.tensor_scalar(
            out=rowbase[:], in0=rowbase[:], scalar1=C, scalar2=None, op0=ALU.mult
        )
        # goff = rowbase + t_i  -> flat element offset of logits[i, t_i]
        nc.vector.tensor_tensor(
            out=goff[:], in0=tlo[:], in1=rowbase[:], op=ALU.add
        )
        # g[p, t] = logits_flat[goff]
        nc.gpsimd.indirect_dma_start(
            out=g[:],
            out_offset=None,
            in_=x.rearrange("r c -> (r c)")[:, None],
            in_offset=bass.IndirectOffsetOnAxis(ap=goff[:], axis=0),
        )

        # --- Streaming loop over row tiles, in four quarters --------------
        # After each quarter finishes, its w-gather / lse / prod / sum tail
        # runs while the next quarter streams, so only the last quarter's
        # (short) tail is exposed at the end.
        qnum = singles.tile([P, 4], F32)
        qden = singles.tile([P, 4], F32)
        xr = x.rearrange("(t p) c -> t p c", p=P)
        cwf = cw.rearrange("a b -> (a b)")[:, None]
        QT = TILES // 4
        EIGHTH = QT // 2
        for h in range(4):
            for e in range(2):
                for t in range(h * QT + e * EIGHTH,
                               h * QT + (e + 1) * EIGHTH):
                    xt = xp.tile([P, C], BF16)
                    nc.sync.dma_start(out=xt[:], in_=xr[t])
                    pk16 = pk16s[t % NPKBUF]
                    nc.scalar.activation(
                        out=pk16[:, :, 1].bitcast(F16), in_=xt[:], func=AF.Exp,
                        accum_out=sumexp_all[:, t : t + 1],
                    )
                    # max8 over the packed tile: v8[0] = max packed value,
                    # whose low 16 bits are the argmax column.
                    nc.vector.max(
                        v8_all[:, t, :], pks[t % NPKBUF][:].bitcast(F32)
                    )

                se = slice(h * QT + e * EIGHTH, h * QT + (e + 1) * EIGHTH)
                # pidx = argmax column = low 16 bits of the packed row max
                pmax_se = v8_all[:, se, 0].bitcast(I32)
                _stt_int(
                    nc, nc.vector, pidx[:, se], pmax_se, 65535,
                    pmax_se, ALU.bitwise_and, ALU.bypass,
                )
                # woff = t_i * 1000 + p_i  (flat offset into class_weights)
                nc.vector.scalar_tensor_tensor(
                    out=woff[:, se], in0=tlo[:, se], scalar=float(C),
                    in1=pidx[:, se], op0=ALU.mult, op1=ALU.add,
                )
                nc.gpsimd.indirect_dma_start(
                    out=w[:, se],
                    out_offset=None,
                    in_=cwf,
                    in_offset=bass.IndirectOffsetOnAxis(ap=woff[:, se], axis=0),
                )

            sl = slice(h * QT, (h + 1) * QT)
            nc.scalar.activation(
                out=lse[:, sl], in_=sumexp_all[:, sl], func=AF.Ln
            )
            nc.vector.tensor_tensor(
                out=diff[:, sl], in0=lse[:, sl], in1=g[:, sl], op=ALU.subtract
            )
            nc.vector.tensor_tensor(
                out=prod[:, sl], in0=w[:, sl], in1=diff[:, sl], op=ALU.mult
            )
            nc.vector.reduce_sum(qnum[:, h : h + 1], prod[:, sl], axis=AX)
            nc.vector.reduce_sum(qden[:, h : h + 1], w[:, sl], axis=AX)

        # --- Tail ----------------------------------------------------------
        nc.vector.reduce_sum(partials[:, 0:1], qnum[:], axis=AX)
        nc.vector.reduce_sum(partials[:, 1:2], qden[:], axis=AX)
        nc.sync.dma_start(out=out[:, :], in_=partials[:])


def build_nc(reps=1):
    """reps>1 repeats the whole computation serially (timing calibration)."""
    nc = bacc.Bacc(None, target_bir_lowering=False)
    x = nc.dram_tensor("x", [R, C], BF16, kind="ExternalInput")
    t32 = nc.dram_tensor("t32", [R, 2], I32, kind="ExternalInput")
    sc32 = nc.dram_tensor("sc32", [C, 2], I32, kind="ExternalInput")
    cw = nc.dram_tensor("cw", [C, C], F32, kind="ExternalInput")
    out = nc.dram_tensor("partials", [P, 2], F32, kind="ExternalOutput")
    with tile.TileContext(nc) as tc:
        for _ in range(reps):
            build_kernel(tc, x, t32, sc32, cw, out)
    nc.compile()
    return nc


_CACHE = {}


def _get_nc():
    if "nc" not in _CACHE:
        _CACHE["nc"] = build_nc()
    return _CACHE["nc"]


def make_in_maps(logits, class_weights, target, super_classes):
    """Shard the full inputs into per-core input maps (host-side: int64
    index tensors are byte-reinterpreted as int32 pairs, and logits are
    downcast to bf16 to halve HBM traffic — the kernel is memory-bound and
    the 2^-9 relative quantization is far inside the 2e-2 tolerance)."""
    import ml_dtypes

    logits = np.asarray(logits, dtype=np.float32).astype(ml_dtypes.bfloat16)
    cw = np.ascontiguousarray(class_weights, dtype=np.float32)
    t32 = (
        np.ascontiguousarray(target, dtype=np.int64)
        .view(np.int32)
        .reshape(N, 2)
    )
    sc32 = (
        np.ascontiguousarray(super_classes, dtype=np.int64)
        .view(np.int32)
        .reshape(C, 2)
    )
    in_maps = []
    for c in range(NCORES):
        sl = slice(c * R, (c + 1) * R)
        in_maps.append(
            {
                "x": np.ascontiguousarray(logits[sl]),
                "t32": np.ascontiguousarray(t32[sl]),
                "sc32": sc32,
                "cw": cw,
            }
        )
    return in_maps


def combine(results):
    num = 0.0
    den = 0.0
    for r in results:
        p = r["partials"].astype(np.float64)
        num += p[:, 0].sum()
        den += p[:, 1].sum()
    return np.asarray(np.float32(num / den))


def kernel(logits, class_weights, target, super_classes, _spmd_kwargs=None):
    nc = _get_nc()
    in_maps = make_in_maps(logits, class_weights, target, super_classes)
    kw = dict(_spmd_kwargs or {})
    res = run_bass_kernel_spmd(nc, in_maps, core_ids=list(range(NCORES)), **kw)
    out = combine(res.results)
    if _spmd_kwargs is not None:
        _CACHE["last_results"] = res
    return out
